# revision 3
# baseline (speedup 1.0000x reference)
"""Multi-head causal attention (B=2, S=2048, E=1024, H=16, D=64) on 8 TRN2
NeuronCores. Sharding: batch (2) x head-groups (4 heads each) -> 8 cores.
Each core computes Q/K/V projections for its 4 heads, RoPE, flash-style
causal attention, and a partial output projection (its head columns of Wo);
the host sums the 4 partials per batch.

Layout/schedule notes:
- Projections and attention are INTERLEAVED per 512-token q-chunk
  (ascending jq): step j projects K/Q/V for chunk j then runs the full
  attention pass for q-chunk j (which only needs chunks <= j). Later
  steps' projection matmuls are emitted as generator "filler" popped
  inside the (exp/ACT-gated) attention inner loop, so the in-order PE
  queue never drains (HAM stays warm). jq=0 pops nothing (its filler's
  inputs may not have landed; a DMA-blocked matmul stalls the queue).
- O-projection is deferred one chunk: its 8 (qb,ec) units pop at the
  next attention pass's hp boundaries, covering the softmax-normalize
  windows where the PE would idle. Per-jq at_c tags make this safe.
- Input DMA rides three queues (sync HWDGE / scalar HWDGE / gpsimd
  SWDGE) in coalesced 0.25-1MB transfers ordered by first use; the
  RoPE cos/sin tables load chunk-0 columns first (critical path into
  attention(0)). Output stores ride the sync ring.
- Scores stationaries are per-head 32-partition DoubleRow slices of the
  RoPE output (tile_position=(32h,0)) -- no zero-padded k copies.
- Causal mask: additive -960 via identity-stationary matmul on just the
  128-col diagonal sub-block (the mask is the same lower-triangle
  [128,128] for every diagonal block).
- Softmax 1/denominator: ln -> exp(-x) on the scalar engine (fp32
  intermediate, ~1.1us chain; the DVE iterative reciprocal costs 3.3us
  per [1,512] row). All activations are forced into the one combined
  natural_log_exp_and_others table set (see get_activation_tables
  patch) -- otherwise the table-load pass thrashes ~1.3us ACT_TABLE_LOADs
  between the exp and ln sets at every boundary.
- exp/ln table set is pre-warmed with a dummy activation during the
  load phase so the one-time ACT_TABLE_LOAD overlaps DMA.
Profiled single-exec: 179us vs 241us for the previous kernel (NTFF
numbers, instrumentation-inflated; rel err 1.590e-02 unchanged).
"""

import sys

if "/opt/trn_rl_repo" not in sys.path:
    sys.path.insert(0, "/opt/trn_rl_repo")

import numpy as np
import ml_dtypes

B, S, E, H = 2, 2048, 1024, 16
D = E // H          # 64
HPC = 4             # heads per core
NCORES = 8
NE = E // 128       # 8 contraction chunks
NQ = S // 512       # 4 q-chunks
NK = S // 128       # 16 k-blocks
ROPE_BASE = 10000.0
ATTN_SCALE = 1.0 / np.sqrt(E)


def build_bass(reps=1):
    import concourse.bass as bass
    import concourse.mybir as mybir
    from concourse import bacc
    from concourse import library_config
    from concourse.tile import TileContext

    F32 = mybir.dt.float32
    BF16 = mybir.dt.bfloat16
    F8 = mybir.dt.float8e4
    DRow = mybir.MatmulPerfMode.DoubleRow
    Exp = mybir.ActivationFunctionType.Exp
    Ln = mybir.ActivationFunctionType.Ln

    # The kernel uses Exp and Ln; left alone, the table-load pass resolves
    # each to the FIRST set containing it (exp_and_others vs natural_log_*)
    # and thrashes ACT_TABLE_LOADs (~1.3us each) at every boundary. Empty
    # out every other set (preserving positions -- act_func_set_id is the
    # index into act_info.json) so both resolve to the one combined set.
    from concourse.hw_specs import get_activation_tables as _orig_gat
    _combined = "natural_log_exp_and_others"
    if _combined in _orig_gat("gen3"):
        bacc.get_activation_tables = lambda arch: {
            k: (v if k == _combined else set())
            for k, v in _orig_gat(arch).items()
        }

    nc = bacc.Bacc()

    U8 = mybir.dt.uint8
    xT_e = nc.declare_dram_parameter("xT", [E, S], BF16, isOutput=False)
    # fp8 operands shipped as uint8 bytes (axon PJRT lacks fp8 buffers):
    # x8: [128, epair, t, S] (p-major so the per-chunk load is a 3-dim AP);
    # wq8/wk8: [epair, 128, t, eo, 128], W pre-scaled by 32 (the 1/32 is
    # folded into the cs/sn RoPE tables).
    x8_e = nc.declare_dram_parameter("x8", [128, 4, 2, S], U8, isOutput=False)
    wq8_e = nc.declare_dram_parameter(
        "wq8", [4, 128, 2, 2, 128], U8, isOutput=False)
    wk8_e = nc.declare_dram_parameter(
        "wk8", [4, 128, 2, 2, 128], U8, isOutput=False)
    wv_e = nc.declare_dram_parameter("wv", [E, 256], BF16, isOutput=False)
    wo_e = nc.declare_dram_parameter("wo", [256, E], BF16, isOutput=False)
    cs_e = nc.declare_dram_parameter("cs", [128, S], BF16, isOutput=False)
    sn_e = nc.declare_dram_parameter("sn", [128, S], BF16, isOutput=False)
    dm_e = nc.declare_dram_parameter("dmask", [128, 128], BF16, isOutput=False)
    id_e = nc.declare_dram_parameter("ident", [128, 128], BF16, isOutput=False)
    out_e = nc.declare_dram_parameter("out", [S, E], BF16, isOutput=True)

    nc.gpsimd.load_library(library_config.attn)

    with TileContext(nc) as tc:
        with (
            tc.tile_pool(name="wpool", bufs=1) as wpool,
            tc.tile_pool(name="xpool", bufs=1) as xpool,
            tc.tile_pool(name="qk", bufs=1) as qkpool,
            tc.tile_pool(name="vpool", bufs=1) as vpool,
            tc.tile_pool(name="epool", bufs=8) as epool,
            tc.tile_pool(name="rtmp", bufs=4) as rtmp,
            tc.tile_pool(name="atp", bufs=1) as atpool,
            tc.tile_pool(name="ypool", bufs=8) as ypool,
            tc.tile_pool(name="npool", bufs=8) as npool,
            tc.tile_pool(name="psA", bufs=2, space="PSUM") as psA,
            tc.tile_pool(name="psS", bufs=2, space="PSUM") as psS,
            tc.tile_pool(name="psO", bufs=1, space="PSUM") as psO,
        ):
          for _rep in range(reps):
            # ---- input DMA: two HWDGE rings, ordered by first use --------
            # sync ring: q/k projection path;  scalar ring: V / mask / Wo.
            wkt = wpool.tile([128, 4, 2, 2, 128], U8, tag="wk", name="wk")
            nc.sync.dma_start(wkt[:], wk8_e.rearrange("r p t e c -> p r t e c"))
            wqt = wpool.tile([128, 4, 2, 2, 128], U8, tag="wq", name="wq")
            nc.sync.dma_start(wqt[:], wq8_e.rearrange("r p t e c -> p r t e c"))
            x8_t = [None] * NQ
            cs_sb = sn_sb = None

            def load_x8(j):
                t = xpool.tile([128, 4, 2, 512], U8, tag=f"x8_{j}",
                               name=f"x8_{j}")
                nc.sync.dma_start(
                    t[:], x8_e[:, :, :, 512 * j: 512 * (j + 1)])
                x8_t[j] = t

            load_x8(0)
            # split the RoPE tables: chunk-0 columns first so the step-0
            # RoPE (the critical path into attention(0)) starts ~4us sooner
            cs_sb = wpool.tile([128, S], BF16, tag="cs")
            nc.sync.dma_start(cs_sb[:, 0:512], cs_e[:, 0:512])
            sn_sb = wpool.tile([128, S], BF16, tag="sn")
            nc.sync.dma_start(sn_sb[:, 0:512], sn_e[:, 0:512])
            dm_sb = wpool.tile([128, 128], BF16, tag="dm")
            nc.sync.dma_start(dm_sb[:], dm_e[:])
            id_sb = wpool.tile([128, 128], BF16, tag="id")
            nc.sync.dma_start(id_sb[:], id_e[:])
            nc.sync.dma_start(cs_sb[:, 512:S], cs_e[:, 512:S])
            nc.sync.dma_start(sn_sb[:, 512:S], sn_e[:, 512:S])
            for j in range(1, NQ):
                load_x8(j)

            xT_t = [None] * NQ

            def load_xT(j):
                t = xpool.tile([128, 8, 512], BF16, tag=f"xT_{j}",
                               name=f"xT_{j}")
                nc.scalar.dma_start(
                    t[:],
                    xT_e[:, 512 * j: 512 * (j + 1)].rearrange(
                        "(c p) s -> p c s", p=128))
                xT_t[j] = t

            # scalar ring carries only what's needed before the first exp
            # (each DMA kick costs ~1.5us of ACT-queue time); dm/id ride
            # the sync ring, later xT chunks ride gpsimd SWDGE.
            load_xT(0)
            wv_sb = wpool.tile([128, 8, 256], BF16, tag="wv", name="wv")
            nc.scalar.dma_start(
                wv_sb[:], wv_e.rearrange("(c p) v -> p c v", p=128))
            wo_sb = wpool.tile([128, 2, E], BF16, tag="wo")
            nc.scalar.dma_start(wo_sb[:], wo_e.rearrange("(c p) e -> p c e", p=128))
            for j in range(1, NQ):
                t = xpool.tile([128, 8, 512], BF16, tag=f"xT_{j}",
                               name=f"xT_{j}")
                nc.gpsimd.dma_start(
                    t[:],
                    xT_e[:, 512 * j: 512 * (j + 1)].rearrange(
                        "(c p) s -> p c s", p=128))
                xT_t[j] = t

            # warm the exp table set during the load phase (one-time ~2.7us
            # ACT_TABLE_LOAD would otherwise stall the first real exp)
            warm = npool.tile([1, 8], BF16, tag="warm")
            warm2 = npool.tile([1, 8], BF16, tag="warm2")
            nc.vector.memset(warm[:], 0.0)
            nc.scalar.activation(warm2[:], warm[:], Exp)

            # ---- per-chunk projections + RoPE -----------------------------
            q8_t, k8_t = ([None] * NQ for _ in range(2))
            v_t = [None] * NK

            def gen_qk(w_t, j, nm, qk_list):
                # generator form: yields between matmul bursts so the caller
                # can interleave these PE ops into the (ACT-gated) attention
                # inner loop of the previous q-chunk.
                sl = slice(512 * j, 512 * (j + 1))
                pe_ps = psA.tile([128, 512], F32, tag="pp", name=f"pe_{nm}{j}")
                po_ps = psA.tile([128, 512], F32, tag="pp", name=f"po_{nm}{j}")
                for pr in range(4):
                    nc.tensor.matmul(
                        pe_ps[:], w_t[:, pr, :, 0, :].bitcast(F8),
                        x8_t[j][:, pr].bitcast(F8),
                        start=(pr == 0), stop=(pr == 3), perf_mode=DRow)
                for pr in range(4):
                    nc.tensor.matmul(
                        po_ps[:], w_t[:, pr, :, 1, :].bitcast(F8),
                        x8_t[j][:, pr].bitcast(F8),
                        start=(pr == 0), stop=(pr == 3), perf_mode=DRow)
                # drain psums to bf16 once, then RoPE in 4x bf16 DVE mode;
                # rotated outputs quantize to fp8e4 in one [128,2,512] tile
                # (dim1 = even/odd) feeding DoubleRow scores matmuls.
                # NOTE: yields only AFTER the psum drain copies are emitted
                # -- other psA users (deferred O-proj units) may run at any
                # yield, and the pool's WAR retirement needs the reads of
                # the previous tile to already be in the stream.
                pe_sb = rtmp.tile([128, 512], BF16, tag="pe_sb", name="pe_sb")
                po_sb = rtmp.tile([128, 512], BF16, tag="po_sb", name="po_sb")
                nc.vector.tensor_copy(pe_sb[:], pe_ps[:])
                nc.vector.tensor_copy(po_sb[:], po_ps[:])
                yield
                t1 = rtmp.tile([128, 512], BF16, tag="t1", name="t1")
                t2 = rtmp.tile([128, 512], BF16, tag="t2", name="t2")
                t3 = rtmp.tile([128, 512], BF16, tag="t3", name="t3")
                t4 = rtmp.tile([128, 512], BF16, tag="t4", name="t4")
                nc.vector.tensor_mul(t1[:], pe_sb[:], cs_sb[:, sl])
                nc.vector.tensor_mul(t2[:], po_sb[:], sn_sb[:, sl])
                nc.vector.tensor_mul(t3[:], pe_sb[:], sn_sb[:, sl])
                nc.vector.tensor_mul(t4[:], po_sb[:], cs_sb[:, sl])
                qk = qkpool.tile([128, 2, 512], F8, tag=f"{nm}8{j}",
                                 name=f"{nm}8{j}")
                with nc.allow_low_precision(reason="fp8 q/k for scores"):
                    nc.vector.tensor_sub(qk[:, 0, :], t1[:], t2[:])
                    nc.vector.tensor_add(qk[:, 1, :], t3[:], t4[:])
                qk_list[j] = qk
                yield

            def gen_v(i):
                # V: natural [S, dims] layout, ones column per head (65 wide)
                pv = psA.tile([128, 256], F32, tag="pp", name=f"pv{i}")
                for e in range(NE):
                    nc.tensor.matmul(
                        pv[:],
                        xT_t[i // 4][:, e, 128 * (i % 4): 128 * (i % 4) + 128],
                        wv_sb[:, e],
                        start=(e == 0), stop=(e == NE - 1))
                vt = vpool.tile([128, 4, 65], BF16, tag=f"v{i}", name=f"v{i}")
                nc.vector.tensor_copy(
                    vt[:, :, 0:64], pv[:].rearrange("p (h d) -> p h d", d=64))
                nc.vector.memset(vt[:, :, 64], 1.0)
                v_t[i] = vt
                yield

            def gen_step(j):
                yield from gen_qk(wkt, j, "k", k8_t)
                yield from gen_qk(wqt, j, "q", q8_t)
                for i in range(4 * j, 4 * j + 4):
                    yield from gen_v(i)

            def oproj_units(jq, at_c):
                # O-projection as 8 independent work units; deferred and
                # popped into the NEXT attention pass's boundary windows
                # (where the PE would otherwise idle behind the softmax
                # normalize chain). Per-jq at_c tags make late reads safe.
                units = []
                for qb in range(4):
                    for ec in range(2):
                        def unit(qb=qb, ec=ec):
                            lsl = slice(128 * qb, 128 * qb + 128)
                            orow = 128 * (4 * jq + qb)
                            esl = slice(512 * ec, 512 * (ec + 1))
                            yp = psA.tile([128, 512], F32, tag="pp")
                            for c in range(2):
                                nc.tensor.matmul(
                                    yp[:], at_c[c][:, lsl], wo_sb[:, c, esl],
                                    start=(c == 0), stop=(c == 1))
                            ys = ypool.tile([128, 512], BF16, tag="y")
                            nc.vector.tensor_copy(ys[:], yp[:])
                            nc.sync.dma_start(
                                out_e[orow: orow + 128, esl], ys[:])
                        units.append(unit)
                return units

            def attention(jq, filler=None, oprev=None):
                nblk = 4 * jq + 4
                at_c = [atpool.tile([128, 512], BF16, tag=f"at{c}_{jq}",
                                    name=f"at{c}_{jq}") for c in range(2)]
                for hp in range(2):
                    po = [psO.tile([65, 512], F32, tag=f"o{g}",
                                   name=f"po{hp}_{g}") for g in range(2)]

                    # software pipeline: emit scores(i+1) BEFORE exp/AV(i) so
                    # the in-order PE queue runs scores(i+1) during exp(i).
                    def emit_scores(i):
                        r = i - 4 * jq
                        q0 = 128 * max(r, 0)
                        jsl = slice(128 * (i % 4), 128 * (i % 4) + 128)
                        ss = psS.tile([128, 2, 512], F32, tag="ss", name="ss")
                        for g in range(2):
                            h = 2 * hp + g
                            hr = slice(32 * h, 32 * h + 32)
                            nc.tensor.matmul(
                                ss[:, g, q0:512],
                                k8_t[i // 4][hr, :, jsl],
                                q8_t[jq][hr, :, q0:512],
                                start=True, stop=(r < 0), perf_mode=DRow,
                                tile_position=(32 * h, 0))
                        if r >= 0:
                            # additive causal mask (-960 -> exp ~ 0) on the
                            # 128-col diagonal sub-block only; identity-
                            # stationary matmul accumulates it into psum.
                            for g in range(2):
                                nc.tensor.matmul(
                                    ss[:, g, q0:q0 + 128], id_sb[:],
                                    dm_sb[:],
                                    start=False, stop=True)
                        return ss

                    def emit_expav(i, ss):
                        r = i - 4 * jq
                        q0 = 128 * max(r, 0)
                        et = epool.tile([128, 2, 512], BF16, tag="e")
                        nc.scalar.activation(
                            et[:, :, q0:512], ss[:, :, q0:512], Exp,
                            scale=ATTN_SCALE)
                        for g in range(2):
                            h = 2 * hp + g
                            nc.tensor.matmul(
                                po[g][:, q0:512], v_t[i][:, h, :],
                                et[:, g, q0:512],
                                start=(i == 0), stop=(i == nblk - 1))

                    prev = None
                    for i in range(nblk):
                        ss = emit_scores(i)
                        if prev is not None:
                            emit_expav(i - 1, prev)
                        # jq=0's inputs for the next chunk may not have
                        # landed yet -- a filler matmul waiting on DMA would
                        # block the in-order PE queue mid-attention.
                        if filler is not None and jq > 0 and i % 3 == 1:
                            next(filler, None)
                        prev = ss
                    emit_expav(nblk - 1, prev)

                    # normalize: at = po[0:64] * (1 / po[64]) -> bf16.
                    # 1/den via ln -> exp(-x) on the scalar engine (fp32
                    # intermediate): ~1.1us chain, no iterative DVE divide.
                    for g in range(2):
                        lnt = npool.tile([1, 512], F32, tag="lnt")
                        nc.scalar.activation(lnt[:], po[g][64:65, :], Ln)
                        rtb = npool.tile([1, 512], BF16, tag="rtb")
                        with nc.allow_low_precision(
                                reason="softmax denom recip in bf16"):
                            nc.scalar.activation(rtb[:], lnt[:], Exp,
                                                 scale=-1.0)
                        bt = npool.tile([64, 512], BF16, tag="bt")
                        nc.gpsimd.partition_broadcast(bt[:], rtb[:])
                        nc.vector.tensor_mul(
                            at_c[hp][64 * g: 64 * g + 64, :],
                            po[g][0:64, :], bt[:])

                    if hp == 0 and oprev:
                        # fill the hp0->hp1 boundary (hp1's first AV waits
                        # on hp0's at-drain via the psO WAR edge) with the
                        # previous chunk's deferred O-projection units.
                        for u in oprev[:4]:
                            u()
                        del oprev[:4]

                # remaining deferred O-proj + leftover projection filler
                # cover the hp1 normalize window before this chunk's own
                # (deferred) O-projection would stall.
                if oprev:
                    for u in oprev:
                        u()
                    oprev.clear()
                if filler is not None:
                    for _ in filler:
                        pass
                return oproj_units(jq, at_c)

            # step 0's projections run as a prologue; each later step's
            # projection matmuls are interleaved into the previous q-chunk's
            # attention pass (whose inner loop is exp/ACT-gated, leaving PE
            # bubbles the projection matmuls fill).
            for _ in gen_step(0):
                pass
            oprev = None
            for j in range(NQ):
                filler = gen_step(j + 1) if j + 1 < NQ else None
                oprev = attention(j, filler, oprev)
            # last chunk's O-projection has no later segment to hide in
            for u in oprev:
                u()
    nc.finalize()
    return nc


def host_inputs(x, Wq, Wk, Wv, Wo):
    """Build the 8 per-core input maps (numpy, host-side shard/permute)."""
    F8 = ml_dtypes.float8_e4m3
    perm = np.concatenate([np.arange(0, D, 2), np.arange(1, D, 2)])  # evens;odds
    d2 = D // 2
    theta = 1.0 / (ROPE_BASE ** (np.arange(d2, dtype=np.float64) * 2.0 / D))
    pos = np.arange(S, dtype=np.float64)
    ang = pos[None, :] * theta[:, None]              # [32, S]
    # q/k psums carry a 32x factor (W pre-scaled into fp8 range); fold the
    # 1/32 into the RoPE tables so the rotated q/k come out at unit scale.
    cs = np.tile(np.cos(ang) / 32.0, (4, 1)).astype(ml_dtypes.bfloat16)
    sn = np.tile(np.sin(ang) / 32.0, (4, 1)).astype(ml_dtypes.bfloat16)

    # additive causal mask for the diagonal 128x128 sub-block: 0 on valid
    # (k <= q) positions, -960 on masked ones (exp(-960/32) ~ 9e-14)
    k_idx = np.arange(128)[:, None]
    c_idx = np.arange(128)[None, :]
    dm = np.where(k_idx <= c_idx, 0.0, -960.0).astype(ml_dtypes.bfloat16)

    def pack_w8(W, ecols, ocols):
        # [E, 2(eo), 128] fp8 of 32*W -> [epair, 128, t, eo, 128] bytes
        w = np.stack([W.T[:, ecols], W.T[:, ocols]], axis=1)
        w8 = (32.0 * w).astype(F8)
        w8 = w8.reshape(4, 2, 128, 2, 128).transpose(0, 2, 1, 3, 4)
        return np.ascontiguousarray(w8).view(np.uint8)

    in_maps = []
    for c in range(NCORES):
        b, g = divmod(c, HPC)
        heads = [HPC * g + t for t in range(HPC)]
        # evens chunk cols: head-major, 32 even dims each; odds chunk likewise
        ecols = np.concatenate([D * h + perm[:d2] for h in heads])
        ocols = np.concatenate([D * h + perm[d2:] for h in heads])
        vcols = np.concatenate([D * h + np.arange(D) for h in heads])
        wv = Wv.T[:, vcols]                                      # [E, 256]
        wo = Wo[:, vcols].T.astype(ml_dtypes.bfloat16)           # [256, E]
        xb = np.ascontiguousarray(x[b].T)                        # [E, S]
        x8 = xb.astype(F8).reshape(4, 2, 128, S).transpose(2, 0, 1, 3)
        in_maps.append({
            "xT": xb.astype(ml_dtypes.bfloat16),
            "x8": np.ascontiguousarray(x8).view(np.uint8),
            "wq8": pack_w8(Wq, ecols, ocols),
            "wk8": pack_w8(Wk, ecols, ocols),
            "wv": np.ascontiguousarray(wv).astype(ml_dtypes.bfloat16),
            "wo": np.ascontiguousarray(wo),
            "cs": cs, "sn": sn, "dmask": dm,
            "ident": np.eye(128, dtype=np.float32).astype(ml_dtypes.bfloat16),
        })
    return in_maps


_CACHED = {}


def kernel(x, Wq, Wk, Wv, Wo):
    from concourse.bass_utils import run_bass_kernel_spmd

    if "nc" not in _CACHED:
        _CACHED["nc"] = build_bass()
    nc = _CACHED["nc"]
    in_maps = host_inputs(
        np.asarray(x, dtype=np.float32), np.asarray(Wq, dtype=np.float32),
        np.asarray(Wk, dtype=np.float32), np.asarray(Wv, dtype=np.float32),
        np.asarray(Wo, dtype=np.float32))
    res = run_bass_kernel_spmd(nc, in_maps, core_ids=list(range(NCORES)))
    y = np.empty((B, S, E), dtype=np.float32)
    for b in range(B):
        y[b] = sum(res.results[HPC * b + g]["out"].astype(np.float32)
                   for g in range(HPC))
    return y
